# revision 6
# baseline (speedup 1.0000x reference)
"""Trainium2 Bass kernel for the memristor-crossbar layer (nn_CustomLayer_30588757082254).

out = unmap(x @ G_eff) + bias, where G_eff = 1/(1/G + R_par) is an elementwise
transform of weight.T with globally min/max-normalized conductances.

Strategy: data-parallel over batch (8 cores x 1024 rows). Each core receives the
full weight.T, its x-slice pre-transposed to [K, 1024] (layout-only host prep,
fed as fp32r bits), and bias. The conductance transform, the 1024x2048x2048
fp32r matmul, the row-sum correction and the bias add all run on device. Host
prep computes only data layout plus the two scalar weight statistics
(wmin/wmax -> s, a, kappa; ~0.01% of the FLOPs) that every transform op
depends on.

Math (S = 1/s folds the output unmapping scale into the transform for free):
  s = (g_max-g_min)/(wmax-wmin);  a = g_min/s - wmin
  S*G = WT + a
  w := u*s = recip(WT+a) + s*R          (R = colvec2 - 256*kt;  colvec2 = 4098+2n-2p)
  geff' := recip(w) = S*G_eff
  out = x@geff' + bias + xs*kappa       (kappa = wmin - g_min*S)

The K accumulation is split in two phases of 8 k-tiles so PSUM groups close as
soon as the first half of the transform is done: phase-1 partials are flushed
PSUM -> SBUF (ACT copy) -> DRAM stage, overlapping phase-2's transform; phase-2
re-loads the stage and the epilogue STT combines psum + beta + stage on DVE.
Row-sums (xs) run on the PE during the phase-1 transform window.
"""
import numpy as np

import concourse.bass as bass
import concourse.mybir as mybir
import concourse.tile as tile
from concourse import bacc
from concourse.bass_utils import run_bass_kernel_spmd
from concourse.dve_ops import RECIP_APPROX_FAST_CONSTS, RECIPROCAL_APPROX_FAST

F32 = mybir.dt.float32
F32R = mybir.dt.float32r
I32 = mybir.dt.int32
AF = mybir.ActivationFunctionType
ALU = mybir.AluOpType
AX = mybir.AxisListType
CRC = RECIP_APPROX_FAST_CONSTS

N_CORES = 8
B, K, N = 8192, 2048, 2048
BC = B // N_CORES            # 1024 batch rows per core
KT = K // 128                # 16 k-tiles
KH = KT // 2                 # k-tiles per phase
MB = BC // 128               # 8 m-blocks per core
NB = N // 512                # 4 psum column chunks
Q = 4                        # transform processed in column quarters of 512

PARASITIC_R = 2.0
G_MIN, G_MAX = 1.0 / 100000.0, 1.0 / 1000.0

_CACHE = {}


def _build_nc():
    nc = bacc.Bacc("TRN2", target_bir_lowering=False, debug=False,
                   num_devices=N_CORES)
    wt_in = nc.dram_tensor("wt", [K, N], F32, kind="ExternalInput")
    xt_in = nc.dram_tensor("xt", [K, BC], F32R, kind="ExternalInput")
    bias_in = nc.dram_tensor("bias", [1, N], F32R, kind="ExternalInput")
    mmx_in = nc.dram_tensor("mmx", [1, 20], F32, kind="ExternalInput")
    out_d = nc.dram_tensor("out", [BC, N], F32, kind="ExternalOutput")
    stage_d = nc.dram_tensor("stage", [BC, N], F32)

    xt_r = xt_in.rearrange("(kt p) m -> p kt m", p=128)

    with tile.TileContext(nc) as tc:
        with (
            tc.tile_pool(name="geffp", bufs=1) as geffp,
            tc.tile_pool(name="wtp", bufs=3) as wtp,
            tc.tile_pool(name="t1p", bufs=2) as t1p,
            tc.tile_pool(name="xmbp", bufs=2) as xmbp,
            tc.tile_pool(name="osbp", bufs=3) as osbp,
            tc.tile_pool(name="stglp", bufs=2) as stglp,
            tc.tile_pool(name="cvp", bufs=1) as cvp,
            tc.tile_pool(name="smallp", bufs=1) as sp,
            tc.tile_pool(name="pcp", bufs=7, space="PSUM") as pcp,
            tc.tile_pool(name="pssp", bufs=1, space="PSUM") as pssp,
        ):
            # -------- colvec2 integer part first (no runtime deps) --------
            cv2s = cvp.tile([128, N], F32, tag="cv2s")
            for q in range(Q):
                cvi = osbp.tile([128, 512], I32, tag="osb", name=f"cvi{q}")
                nc.gpsimd.iota(cvi[:], pattern=[[2, 512]],
                               base=4098 + 1024 * q, channel_multiplier=-2)
                nc.vector.tensor_copy(cv2s[:, q * 512:(q + 1) * 512], cvi[:])

            # ---------------- tiny inputs + broadcast ----------------
            with nc.named_scope("setup"):
                mmx = sp.tile([1, 20], F32, tag="mmx")
                nc.sync.dma_start(out=mmx[:], in_=mmx_in[:])
                bias_row = sp.tile([1, N], F32R, tag="bias_row")
                nc.sync.dma_start(out=bias_row[:], in_=bias_in[:])
                ones_f = sp.tile([1, 128], F32, tag="ones_f")
                nc.vector.memset(ones_f[:], 1.0)
                ps_bc = pssp.tile([128, 20], F32, tag="pss")
                nc.tensor.matmul(ps_bc[:], ones_f[:], mmx[:], start=True,
                                 stop=True)
                bcv = sp.tile([128, 20], F32, tag="bcv")
                nc.vector.tensor_copy(bcv[:], ps_bc[:])
                ones_col_f = sp.tile([128, 2], F32, tag="ones_col_f")
                nc.vector.memset(ones_col_f[:], 1.0)
                ones_col = sp.tile([128, 2], F32R, tag="ones_col")
                nc.vector.tensor_copy(ones_col[:], ones_col_f[:])
                ones_row_f = sp.tile([1, 128], F32, tag="ones_row_f")
                nc.vector.memset(ones_row_f[:], 1.0)
                ones_row = sp.tile([1, 128], F32R, tag="ones_row")
                nc.vector.tensor_copy(ones_row[:], ones_row_f[:])
            s_b = bcv[:, 0:1]
            a_b = bcv[:, 1:2]
            kap_b = bcv[:, 2:3]
            rk_s = bcv[:, 4:20]
            # scale colvec by s
            nc.vector.tensor_scalar(cv2s[:], cv2s[:], s_b, None, ALU.mult)

            # ---------------- W DMA stream (halves, shared slots) ------------
            wt_t = {}
            for kt in range(KT):
                for h in range(2):
                    w_t = wtp.tile([128, 1024], F32, tag="wt", name=f"wt{kt}_{h}")
                    nc.sync.dma_start(
                        out=w_t[:],
                        in_=wt_in[kt * 128:(kt + 1) * 128,
                                  h * 1024:(h + 1) * 1024])
                    wt_t[kt, h] = w_t

            # --------- xs row-sum groups (PE work for the early window) ------
            xmbs = {}
            xs_sb = sp.tile([128, MB], F32, tag="xs_sb")
            xs_copies = []          # deferred ACT copies, interleaved below
            with nc.named_scope("xs"):
                for mb in range(MB):
                    xmb = xmbp.tile([128, KT, 128], F32R, tag="xmb",
                                    name=f"xmb1_{mb}")
                    nc.gpsimd.dma_start(out=xmb[:],
                                        in_=xt_r[:, :, mb * 128:(mb + 1) * 128])
                    xmbs[1, mb] = xmb
                    ps_xs = pssp.tile([128, 2], F32, tag="pss",
                                      name=f"psxs{mb}")
                    for kt in range(KT):
                        nc.tensor.matmul(ps_xs[:], xmb[:, kt, :], ones_col[:],
                                         start=(kt == 0), stop=(kt == KT - 1))
                    xs_copies.append((mb, ps_xs))

            def transform_tile(kt):
                """geff'[kt] = recip(recip(WT+a) + s*R), in 512-col quarters."""
                ge = geffp.tile([128, N], F32R, tag=f"ge{kt}", name=f"ge{kt}")
                for q in range(Q):
                    h, qs = q // 2, q * 512
                    ws = qs - h * 1024
                    t1 = t1p.tile([128, 512], F32, tag="t1", name=f"t1_{kt}_{q}")
                    nc.scalar.activation(t1[:], wt_t[kt, h][:, ws:ws + 512],
                                         AF.Identity, bias=a_b, scale=1.0)
                    nc.vector._custom_dve(RECIPROCAL_APPROX_FAST, out=t1[:],
                                          in0=t1[:], s0=CRC["s0"],
                                          s1=CRC["s1"], imm2=CRC["imm2"])
                    nc.vector.scalar_tensor_tensor(
                        t1[:], t1[:], rk_s[:, kt:kt + 1],
                        cv2s[:, qs:qs + 512], ALU.add, ALU.add)
                    nc.vector._custom_dve(RECIPROCAL_APPROX_FAST,
                                          out=ge[:, qs:qs + 512], in0=t1[:],
                                          s0=CRC["s0"], s1=CRC["s1"],
                                          imm2=CRC["imm2"])
                # park one xs result per transform tile (keeps ACT flowing and
                # frees the single xs PSUM slot at the PE's pace)
                if xs_copies:
                    mb, ps_xs = xs_copies.pop(0)
                    nc.scalar.copy(xs_sb[:, mb:mb + 1], ps_xs[:, 0:1])
                return ge

            # ---------------- phase 1 transform ----------------
            geff = {}
            with nc.named_scope("transform1"):
                for kt in range(KH):
                    geff[kt] = transform_tile(kt)

            # ------- phase 2 transform interleaved with phase-1 matmuls ------
            with nc.named_scope("p2t_mm1"):
                for j in range(MB):
                    geff[KH + j] = transform_tile(KH + j)
                    mb = j
                    pcs = [pcp.tile([128, 512], F32, tag="pc",
                                    name=f"p1_{mb}_{nb}") for nb in range(NB)]
                    for kt in range(KH):
                        for nb in range(NB):
                            nc.tensor.matmul(
                                pcs[nb][:], xmbs[1, mb][:, kt, :],
                                geff[kt][:, nb * 512:(nb + 1) * 512],
                                start=(kt == 0), stop=(kt == KH - 1))
                    for nb in range(NB):
                        fl = osbp.tile([128, 512], F32, tag="osb",
                                       name=f"fl_{mb}_{nb}")
                        nc.scalar.copy(fl[:], pcs[nb][:])
                        nc.scalar.dma_start(
                            out=stage_d[mb * 128:(mb + 1) * 128,
                                        nb * 512:(nb + 1) * 512],
                            in_=fl[:])

            # betas for the epilogue
            betas = sp.tile([128, MB], F32, tag="betas")
            nc.vector.tensor_scalar(betas[:], xs_sb[:], kap_b, None, ALU.mult)

            # ---------------- phase 2 matmuls + epilogue ----------------
            with nc.named_scope("mm2"):
                for mb in range(MB):
                    xmb = xmbp.tile([128, KH, 128], F32R, tag="xmb",
                                    name=f"xmb2_{mb}")
                    nc.gpsimd.dma_start(
                        out=xmb[:],
                        in_=xt_r[:, KH:KT, mb * 128:(mb + 1) * 128])
                    stgl = [stglp.tile([128, 1024], F32, tag="stgl",
                                       name=f"stgl_{mb}_{h}") for h in range(2)]
                    for h in range(2):
                        nc.scalar.dma_start(
                            out=stgl[h][:],
                            in_=stage_d[mb * 128:(mb + 1) * 128,
                                        h * 1024:(h + 1) * 1024])
                    pcs = [pcp.tile([128, 512], F32, tag="pc",
                                    name=f"p2_{mb}_{nb}") for nb in range(NB)]
                    # bias row opens each accumulation group
                    for nb in range(NB):
                        nc.tensor.matmul(pcs[nb][:], ones_row[:],
                                         bias_row[:, nb * 512:(nb + 1) * 512],
                                         start=True, stop=False)
                    for kt in range(KH):
                        for nb in range(NB):
                            nc.tensor.matmul(
                                pcs[nb][:], xmb[:, kt, :],
                                geff[KH + kt][:, nb * 512:(nb + 1) * 512],
                                start=False, stop=(kt == KH - 1))
                    for nb in range(NB):
                        osb = osbp.tile([128, 512], F32, tag="osb",
                                        name=f"ep_{mb}_{nb}")
                        nc.vector.scalar_tensor_tensor(
                            osb[:], pcs[nb][:], betas[:, mb:mb + 1],
                            stgl[nb // 2][:, (nb % 2) * 512:(nb % 2 + 1) * 512],
                            ALU.add, ALU.add)
                        nc.sync.dma_start(
                            out=out_d[mb * 128:(mb + 1) * 128,
                                      nb * 512:(nb + 1) * 512],
                            in_=osb[:])
    nc.finalize()
    return nc


def _prep_inputs(x, weight, bias):
    wtT = np.ascontiguousarray(weight.T)          # [K, N]
    wmin = float(wtT.min())
    wmax = float(wtT.max())
    s = (G_MAX - G_MIN) / (wmax - wmin)
    a = G_MIN / s - wmin
    kappa = wmin - G_MIN / s
    mmx = np.zeros((1, 20), dtype=np.float32)
    mmx[0, 0] = s
    mmx[0, 1] = a
    mmx[0, 2] = kappa
    mmx[0, 4:20] = [-256.0 * kt * s for kt in range(KT)]

    bias2 = np.ascontiguousarray(bias.reshape(1, N)).astype(np.float32)
    in_maps = []
    for c in range(N_CORES):
        x_c = x[c * BC:(c + 1) * BC, :]           # [BC, K]
        xt_c = np.ascontiguousarray(x_c.T)
        in_maps.append({"wt": wtT, "xt": xt_c, "bias": bias2, "mmx": mmx})
    return in_maps


def _run(x, weight, bias, trace=False, trace_kwargs=None):
    if "nc" not in _CACHE:
        _CACHE["nc"] = _build_nc()
    nc = _CACHE["nc"]
    in_maps = _prep_inputs(x, weight, bias)
    res = run_bass_kernel_spmd(nc, in_maps, list(range(N_CORES)), trace=trace,
                               **(trace_kwargs or {}))
    out = np.concatenate([res.results[c]["out"] for c in range(N_CORES)], axis=0)
    return out, res


def kernel(x, weight, bias):
    x = np.asarray(x, dtype=np.float32)
    weight = np.asarray(weight, dtype=np.float32)
    bias = np.asarray(bias, dtype=np.float32)
    out, _ = _run(x, weight, bias, trace=False)
    return out.astype(np.float32)


# revision 7
# speedup vs baseline: 1.1379x; 1.1379x over previous
"""Trainium2 Bass kernel for the memristor-crossbar layer (nn_CustomLayer_30588757082254).

out = unmap(x @ G_eff) + bias, where G_eff = 1/(1/G + R_par) is an elementwise
transform of weight.T with globally min/max-normalized conductances.

Strategy: data-parallel over batch (8 cores x 1024 rows). Each core receives the
full weight.T, its x-slice pre-transposed to [K, 1024] (layout-only host prep,
fed as fp32r bits), and bias. The conductance transform, the 1024x2048x2048
fp32r matmul, the row-sum correction and the bias add all run on device. Host
prep computes only data layout plus the two scalar weight statistics
(wmin/wmax -> s, a, kappa; ~0.01% of the FLOPs) that every transform op
depends on.

Math (S = 1/s folds the output unmapping scale into the transform for free):
  s = (g_max-g_min)/(wmax-wmin);  a = g_min/s - wmin
  S*G = WT + a
  w := u*s = recip(WT+a) + s*R          (R = colvec2 - 256*kt;  colvec2 = 4098+2n-2p)
  geff' := recip(w) = S*G_eff
  out = x@geff' + bias + xs*kappa       (kappa = wmin - g_min*S)

The K accumulation is split in two phases of 8 k-tiles so PSUM groups close as
soon as the first half of the transform is done: phase-1 partials are flushed
PSUM -> SBUF (ACT copy) -> DRAM stage, overlapping phase-2's transform; phase-2
re-loads the stage and the epilogue STT combines psum + beta + stage on DVE.
Row-sums (xs) run on the PE during the phase-1 transform window.
"""
import numpy as np

import concourse.bass as bass
import concourse.mybir as mybir
import concourse.tile as tile
from concourse import bacc
from concourse.bass_utils import run_bass_kernel_spmd
from concourse.dve_ops import RECIP_APPROX_FAST_CONSTS, RECIPROCAL_APPROX_FAST

F32 = mybir.dt.float32
F32R = mybir.dt.float32r
I32 = mybir.dt.int32
AF = mybir.ActivationFunctionType
ALU = mybir.AluOpType
AX = mybir.AxisListType
CRC = RECIP_APPROX_FAST_CONSTS

N_CORES = 8
B, K, N = 8192, 2048, 2048
BC = B // N_CORES            # 1024 batch rows per core
KT = K // 128                # 16 k-tiles
KH = KT // 2                 # k-tiles per phase
MB = BC // 128               # 8 m-blocks per core
NB = N // 512                # 4 psum column chunks
Q = 4                        # transform processed in column quarters of 512

PARASITIC_R = 2.0
G_MIN, G_MAX = 1.0 / 100000.0, 1.0 / 1000.0

_CACHE = {}


def _build_nc():
    nc = bacc.Bacc("TRN2", target_bir_lowering=False, debug=False,
                   num_devices=N_CORES)
    wt_in = nc.dram_tensor("wt", [K, N], F32, kind="ExternalInput")
    xt_in = nc.dram_tensor("xt", [K, BC], F32R, kind="ExternalInput")
    bias_in = nc.dram_tensor("bias", [1, N], F32R, kind="ExternalInput")
    mmx_in = nc.dram_tensor("mmx", [1, 20], F32, kind="ExternalInput")
    out_d = nc.dram_tensor("out", [BC, N], F32, kind="ExternalOutput")
    stage_d = nc.dram_tensor("stage", [BC, N], F32)

    xt_r = xt_in.rearrange("(kt p) m -> p kt m", p=128)

    with tile.TileContext(nc) as tc:
        with (
            tc.tile_pool(name="geffp", bufs=1) as geffp,
            tc.tile_pool(name="wtp", bufs=3) as wtp,
            tc.tile_pool(name="t1p", bufs=2) as t1p,
            tc.tile_pool(name="xmbp", bufs=2) as xmbp,
            tc.tile_pool(name="osbp", bufs=3) as osbp,
            tc.tile_pool(name="stglp", bufs=2) as stglp,
            tc.tile_pool(name="cvp", bufs=1) as cvp,
            tc.tile_pool(name="smallp", bufs=1) as sp,
            tc.tile_pool(name="pcp", bufs=6, space="PSUM") as pcp,
            tc.tile_pool(name="pssp", bufs=2, space="PSUM") as pssp,
        ):
            # -------- colvec2 integer part first (no runtime deps) --------
            cv2s = cvp.tile([128, N], F32, tag="cv2s")
            for q in range(Q):
                cvi = osbp.tile([128, 512], I32, tag="osb", name=f"cvi{q}")
                nc.gpsimd.iota(cvi[:], pattern=[[2, 512]],
                               base=4098 + 1024 * q, channel_multiplier=-2)
                nc.vector.tensor_copy(cv2s[:, q * 512:(q + 1) * 512], cvi[:])

            # ---------------- tiny inputs + broadcast ----------------
            with nc.named_scope("setup"):
                mmx = sp.tile([1, 20], F32, tag="mmx")
                nc.sync.dma_start(out=mmx[:], in_=mmx_in[:])
                bias_row = sp.tile([1, N], F32R, tag="bias_row")
                nc.sync.dma_start(out=bias_row[:], in_=bias_in[:])
                ones_f = sp.tile([1, 128], F32, tag="ones_f")
                nc.vector.memset(ones_f[:], 1.0)
                ps_bc = pssp.tile([128, 20], F32, tag="pss")
                nc.tensor.matmul(ps_bc[:], ones_f[:], mmx[:], start=True,
                                 stop=True)
                bcv = sp.tile([128, 20], F32, tag="bcv")
                nc.vector.tensor_copy(bcv[:], ps_bc[:])
                ones_col_f = sp.tile([128, 2], F32, tag="ones_col_f")
                nc.vector.memset(ones_col_f[:], 1.0)
                ones_col = sp.tile([128, 2], F32R, tag="ones_col")
                nc.vector.tensor_copy(ones_col[:], ones_col_f[:])
                ones_row_f = sp.tile([1, 128], F32, tag="ones_row_f")
                nc.vector.memset(ones_row_f[:], 1.0)
                ones_row = sp.tile([1, 128], F32R, tag="ones_row")
                nc.vector.tensor_copy(ones_row[:], ones_row_f[:])
            s_b = bcv[:, 0:1]
            a_b = bcv[:, 1:2]
            kap_b = bcv[:, 2:3]
            rk_s = bcv[:, 4:20]
            # scale colvec by s
            nc.vector.tensor_scalar(cv2s[:], cv2s[:], s_b, None, ALU.mult)

            # ---------------- W DMA stream (halves, shared slots) ------------
            wt_t = {}
            for kt in range(KT):
                for h in range(2):
                    w_t = wtp.tile([128, 1024], F32, tag="wt", name=f"wt{kt}_{h}")
                    nc.sync.dma_start(
                        out=w_t[:],
                        in_=wt_in[kt * 128:(kt + 1) * 128,
                                  h * 1024:(h + 1) * 1024])
                    wt_t[kt, h] = w_t

            xmbs = {}
            def transform_tile(kt):
                """geff'[kt] = recip(recip(WT+a) + s*R), in 512-col quarters."""
                ge = geffp.tile([128, N], F32R, tag=f"ge{kt}", name=f"ge{kt}")
                for q in range(Q):
                    h, qs = q // 2, q * 512
                    ws = qs - h * 1024
                    t1 = t1p.tile([128, 512], F32, tag="t1", name=f"t1_{kt}_{q}")
                    nc.scalar.activation(t1[:], wt_t[kt, h][:, ws:ws + 512],
                                         AF.Identity, bias=a_b, scale=1.0)
                    nc.vector._custom_dve(RECIPROCAL_APPROX_FAST, out=t1[:],
                                          in0=t1[:], s0=CRC["s0"],
                                          s1=CRC["s1"], imm2=CRC["imm2"])
                    nc.vector.scalar_tensor_tensor(
                        t1[:], t1[:], rk_s[:, kt:kt + 1],
                        cv2s[:, qs:qs + 512], ALU.add, ALU.add)
                    nc.vector._custom_dve(RECIPROCAL_APPROX_FAST,
                                          out=ge[:, qs:qs + 512], in0=t1[:],
                                          s0=CRC["s0"], s1=CRC["s1"],
                                          imm2=CRC["imm2"])
                return ge

            # ---------------- phase 1 transform ----------------
            geff = {}
            with nc.named_scope("transform1"):
                for kt in range(KH):
                    geff[kt] = transform_tile(kt)

            # ------- phase 2 transform interleaved with phase-1 matmuls ------
            with nc.named_scope("p2t_mm1"):
                for j in range(MB):
                    geff[KH + j] = transform_tile(KH + j)
                    mb = j
                    xmb1 = xmbp.tile([128, KH, 128], F32R, tag="xmb",
                                     name=f"xmb1_{mb}")
                    nc.gpsimd.dma_start(
                        out=xmb1[:],
                        in_=xt_r[:, 0:KH, mb * 128:(mb + 1) * 128])
                    xmbs[1, mb] = xmb1
                    pcs = [pcp.tile([128, 512], F32, tag="pc",
                                    name=f"p1_{mb}_{nb}") for nb in range(NB)]
                    for kt in range(KH):
                        for nb in range(NB):
                            nc.tensor.matmul(
                                pcs[nb][:], xmbs[1, mb][:, kt, :],
                                geff[kt][:, nb * 512:(nb + 1) * 512],
                                start=(kt == 0), stop=(kt == KH - 1))
                    for nb in range(NB):
                        fl = osbp.tile([128, 512], F32, tag="osb",
                                       name=f"fl_{mb}_{nb}")
                        nc.scalar.copy(fl[:], pcs[nb][:])
                        nc.scalar.dma_start(
                            out=stage_d[mb * 128:(mb + 1) * 128,
                                        nb * 512:(nb + 1) * 512],
                            in_=fl[:])

            # ---------------- phase 2 matmuls + epilogue ----------------
            with nc.named_scope("mm2"):
                for mb in range(MB):
                    xmb = xmbp.tile([128, KT, 128], F32R, tag="xmb",
                                    name=f"xmb2_{mb}")
                    nc.gpsimd.dma_start(out=xmb[:],
                                        in_=xt_r[:, :, mb * 128:(mb + 1) * 128])
                    stgl = [stglp.tile([128, 1024], F32, tag="stgl",
                                       name=f"stgl_{mb}_{h}") for h in range(2)]
                    for h in range(2):
                        nc.scalar.dma_start(
                            out=stgl[h][:],
                            in_=stage_d[mb * 128:(mb + 1) * 128,
                                        h * 1024:(h + 1) * 1024])
                    ps_xs = pssp.tile([128, 2], F32, tag="pss",
                                      name=f"psxs{mb}")
                    pcs = [pcp.tile([128, 512], F32, tag="pc",
                                    name=f"p2_{mb}_{nb}") for nb in range(NB)]
                    # bias row opens each accumulation group
                    for nb in range(NB):
                        nc.tensor.matmul(pcs[nb][:], ones_row[:],
                                         bias_row[:, nb * 512:(nb + 1) * 512],
                                         start=True, stop=False)
                    for kt in range(KT):
                        nc.tensor.matmul(ps_xs[:], xmb[:, kt, :], ones_col[:],
                                         start=(kt == 0), stop=(kt == KT - 1))
                        if kt >= KH:
                            for nb in range(NB):
                                nc.tensor.matmul(
                                    pcs[nb][:], xmb[:, kt, :],
                                    geff[kt][:, nb * 512:(nb + 1) * 512],
                                    start=False, stop=(kt == KT - 1))
                    beta = sp.tile([128, 1], F32, tag=f"beta{mb}",
                                   name=f"beta{mb}")
                    nc.vector.tensor_scalar(beta[:], ps_xs[:, 0:1], kap_b, None,
                                            ALU.mult)
                    for nb in range(NB):
                        osb = osbp.tile([128, 512], F32, tag="osb",
                                        name=f"ep_{mb}_{nb}")
                        nc.vector.scalar_tensor_tensor(
                            osb[:], pcs[nb][:], beta[:],
                            stgl[nb // 2][:, (nb % 2) * 512:(nb % 2 + 1) * 512],
                            ALU.add, ALU.add)
                        nc.sync.dma_start(
                            out=out_d[mb * 128:(mb + 1) * 128,
                                      nb * 512:(nb + 1) * 512],
                            in_=osb[:])
    nc.finalize()
    return nc


def _prep_inputs(x, weight, bias):
    wtT = np.ascontiguousarray(weight.T)          # [K, N]
    wmin = float(wtT.min())
    wmax = float(wtT.max())
    s = (G_MAX - G_MIN) / (wmax - wmin)
    a = G_MIN / s - wmin
    kappa = wmin - G_MIN / s
    mmx = np.zeros((1, 20), dtype=np.float32)
    mmx[0, 0] = s
    mmx[0, 1] = a
    mmx[0, 2] = kappa
    mmx[0, 4:20] = [-256.0 * kt * s for kt in range(KT)]

    bias2 = np.ascontiguousarray(bias.reshape(1, N)).astype(np.float32)
    in_maps = []
    for c in range(N_CORES):
        x_c = x[c * BC:(c + 1) * BC, :]           # [BC, K]
        xt_c = np.ascontiguousarray(x_c.T)
        in_maps.append({"wt": wtT, "xt": xt_c, "bias": bias2, "mmx": mmx})
    return in_maps


def _run(x, weight, bias, trace=False, trace_kwargs=None):
    if "nc" not in _CACHE:
        _CACHE["nc"] = _build_nc()
    nc = _CACHE["nc"]
    in_maps = _prep_inputs(x, weight, bias)
    res = run_bass_kernel_spmd(nc, in_maps, list(range(N_CORES)), trace=trace,
                               **(trace_kwargs or {}))
    out = np.concatenate([res.results[c]["out"] for c in range(N_CORES)], axis=0)
    return out, res


def kernel(x, weight, bias):
    x = np.asarray(x, dtype=np.float32)
    weight = np.asarray(weight, dtype=np.float32)
    bias = np.asarray(bias, dtype=np.float32)
    out, _ = _run(x, weight, bias, trace=False)
    return out.astype(np.float32)


# revision 8
# speedup vs baseline: 1.2214x; 1.0733x over previous
"""Trainium2 Bass kernel for the memristor-crossbar layer (nn_CustomLayer_30588757082254).

out = unmap(x @ G_eff) + bias, where G_eff = 1/(1/G + R_par) is an elementwise
transform of weight.T with globally min/max-normalized conductances.

Strategy: data-parallel over batch (8 cores x 1024 rows). Each core receives the
full weight.T, its x-slice pre-transposed to [K, 1024] (layout-only host prep,
fed as fp32r bits), and bias. The conductance transform, the 1024x2048x2048
fp32r matmul, the row-sum correction and the bias add all run on device. Host
prep computes only data layout plus the two scalar weight statistics
(wmin/wmax -> s, a, kappa; ~0.01% of the FLOPs) that every transform op
depends on.

Math (S = 1/s folds the output unmapping scale into the transform for free):
  s = (g_max-g_min)/(wmax-wmin);  a = g_min/s - wmin
  S*G = WT + a
  w := u*s = recip(WT+a) + s*R          (R = colvec2 - 256*kt;  colvec2 = 4098+2n-2p)
  geff' := recip(w) = S*G_eff
  out = x@geff' + bias + xs*kappa       (kappa = wmin - g_min*S)

The K accumulation is split in two phases of 8 k-tiles so PSUM groups close as
soon as the first half of the transform is done: phase-1 partials are flushed
PSUM -> SBUF (ACT copy) -> DRAM stage, overlapping phase-2's transform; phase-2
re-loads the stage and the epilogue STT combines psum + beta + stage on DVE.
Row-sums (xs) run on the PE during the phase-1 transform window.
"""
import numpy as np

import concourse.bass as bass
import concourse.mybir as mybir
import concourse.tile as tile
from concourse import bacc
from concourse.bass_utils import run_bass_kernel_spmd
from concourse.dve_ops import RECIP_APPROX_FAST_CONSTS, RECIPROCAL_APPROX_FAST

F32 = mybir.dt.float32
F32R = mybir.dt.float32r
I32 = mybir.dt.int32
AF = mybir.ActivationFunctionType
ALU = mybir.AluOpType
AX = mybir.AxisListType
CRC = RECIP_APPROX_FAST_CONSTS

N_CORES = 8
B, K, N = 8192, 2048, 2048
BC = B // N_CORES            # 1024 batch rows per core
KT = K // 128                # 16 k-tiles
KH = KT // 2                 # k-tiles per phase
MB = BC // 128               # 8 m-blocks per core
NB = N // 512                # 4 psum column chunks
Q = 4                        # transform processed in column quarters of 512

PARASITIC_R = 2.0
G_MIN, G_MAX = 1.0 / 100000.0, 1.0 / 1000.0

_CACHE = {}


def _build_nc():
    nc = bacc.Bacc("TRN2", target_bir_lowering=False, debug=False,
                   num_devices=N_CORES)
    wt_in = nc.dram_tensor("wt", [K, N], F32, kind="ExternalInput")
    xt_in = nc.dram_tensor("xt", [K, BC], F32R, kind="ExternalInput")
    bias_in = nc.dram_tensor("bias", [1, N], F32R, kind="ExternalInput")
    mmx_in = nc.dram_tensor("mmx", [1, 20], F32, kind="ExternalInput")
    out_d = nc.dram_tensor("out", [BC, N], F32, kind="ExternalOutput")
    stage_d = nc.dram_tensor("stage", [BC, N], F32)

    xt_r = xt_in.rearrange("(kt p) m -> p kt m", p=128)

    with tile.TileContext(nc) as tc:
        with (
            tc.tile_pool(name="geffp", bufs=1) as geffp,
            tc.tile_pool(name="wtp", bufs=3) as wtp,
            tc.tile_pool(name="t1p", bufs=2) as t1p,
            tc.tile_pool(name="xmbp", bufs=2) as xmbp,
            tc.tile_pool(name="osbp", bufs=3) as osbp,
            tc.tile_pool(name="stglp", bufs=2) as stglp,
            tc.tile_pool(name="cvp", bufs=1) as cvp,
            tc.tile_pool(name="smallp", bufs=1) as sp,
            tc.tile_pool(name="pcp", bufs=6, space="PSUM") as pcp,
            tc.tile_pool(name="pssp", bufs=2, space="PSUM") as pssp,
        ):
            # -------- colvec2 integer part first (no runtime deps) --------
            cv2s = cvp.tile([128, N], F32, tag="cv2s")
            for q in range(Q):
                cvi = osbp.tile([128, 512], I32, tag="osb", name=f"cvi{q}")
                nc.gpsimd.iota(cvi[:], pattern=[[2, 512]],
                               base=4098 + 1024 * q, channel_multiplier=-2)
                nc.vector.tensor_copy(cv2s[:, q * 512:(q + 1) * 512], cvi[:])

            # ---------------- tiny inputs + broadcast ----------------
            with nc.named_scope("setup"):
                mmx = sp.tile([1, 20], F32, tag="mmx")
                nc.sync.dma_start(out=mmx[:], in_=mmx_in[:])
                bias_row = sp.tile([1, N], F32R, tag="bias_row")
                nc.sync.dma_start(out=bias_row[:], in_=bias_in[:])
                ones_f = sp.tile([1, 128], F32, tag="ones_f")
                nc.vector.memset(ones_f[:], 1.0)
                ps_bc = pssp.tile([128, 20], F32, tag="pss")
                nc.tensor.matmul(ps_bc[:], ones_f[:], mmx[:], start=True,
                                 stop=True)
                bcv = sp.tile([128, 20], F32, tag="bcv")
                nc.vector.tensor_copy(bcv[:], ps_bc[:])
                ones_col_f = sp.tile([128, 2], F32, tag="ones_col_f")
                nc.vector.memset(ones_col_f[:], 1.0)
                ones_col = sp.tile([128, 2], F32R, tag="ones_col")
                nc.vector.tensor_copy(ones_col[:], ones_col_f[:])
                ones_row_f = sp.tile([1, 128], F32, tag="ones_row_f")
                nc.vector.memset(ones_row_f[:], 1.0)
                ones_row = sp.tile([1, 128], F32R, tag="ones_row")
                nc.vector.tensor_copy(ones_row[:], ones_row_f[:])
            s_b = bcv[:, 0:1]
            a_b = bcv[:, 1:2]
            kap_b = bcv[:, 2:3]
            rk_s = bcv[:, 4:20]
            # scale colvec by s
            nc.vector.tensor_scalar(cv2s[:], cv2s[:], s_b, None, ALU.mult)

            # ---------------- W DMA stream (halves, shared slots) ------------
            wt_t = {}
            for kt in range(KT):
                for h in range(2):
                    w_t = wtp.tile([128, 1024], F32, tag="wt", name=f"wt{kt}_{h}")
                    nc.sync.dma_start(
                        out=w_t[:],
                        in_=wt_in[kt * 128:(kt + 1) * 128,
                                  h * 1024:(h + 1) * 1024])
                    wt_t[kt, h] = w_t

            xmbs = {}
            def transform_tile(kt):
                """geff'[kt] = recip(recip(WT+a) + s*R), in 512-col quarters."""
                ge = geffp.tile([128, N], F32R, tag=f"ge{kt}", name=f"ge{kt}")
                for q in range(Q):
                    h, qs = q // 2, q * 512
                    ws = qs - h * 1024
                    t1 = t1p.tile([128, 512], F32, tag="t1", name=f"t1_{kt}_{q}")
                    nc.scalar.activation(t1[:], wt_t[kt, h][:, ws:ws + 512],
                                         AF.Identity, bias=a_b, scale=1.0)
                    nc.vector._custom_dve(RECIPROCAL_APPROX_FAST, out=t1[:],
                                          in0=t1[:], s0=CRC["s0"],
                                          s1=CRC["s1"], imm2=CRC["imm2"])
                    nc.vector.scalar_tensor_tensor(
                        t1[:], t1[:], rk_s[:, kt:kt + 1],
                        cv2s[:, qs:qs + 512], ALU.add, ALU.add)
                    nc.vector._custom_dve(RECIPROCAL_APPROX_FAST,
                                          out=ge[:, qs:qs + 512], in0=t1[:],
                                          s0=CRC["s0"], s1=CRC["s1"],
                                          imm2=CRC["imm2"])
                return ge

            # ---------------- phase 1 transform ----------------
            geff = {}
            with nc.named_scope("transform1"):
                for kt in range(KH):
                    geff[kt] = transform_tile(kt)

            # ------- phase 2 transform interleaved with phase-1 matmuls ------
            with nc.named_scope("p2t_mm1"):
                for j in range(MB):
                    geff[KH + j] = transform_tile(KH + j)
                    mb = j
                    xmb1 = xmbp.tile([128, KH, 128], F32R, tag="xmb",
                                     name=f"xmb1_{mb}")
                    nc.gpsimd.dma_start(
                        out=xmb1[:],
                        in_=xt_r[:, 0:KH, mb * 128:(mb + 1) * 128])
                    xmbs[1, mb] = xmb1
                    for nb in range(NB):
                        pc = pcp.tile([128, 512], F32, tag="pc",
                                      name=f"p1_{mb}_{nb}")
                        for kt in range(KH):
                            nc.tensor.matmul(
                                pc[:], xmbs[1, mb][:, kt, :],
                                geff[kt][:, nb * 512:(nb + 1) * 512],
                                start=(kt == 0), stop=(kt == KH - 1))
                        fl = osbp.tile([128, 512], F32, tag="osb",
                                       name=f"fl_{mb}_{nb}")
                        nc.scalar.copy(fl[:], pc[:])
                        nc.scalar.dma_start(
                            out=stage_d[mb * 128:(mb + 1) * 128,
                                        nb * 512:(nb + 1) * 512],
                            in_=fl[:])

            # ---------------- phase 2 matmuls + epilogue ----------------
            with nc.named_scope("mm2"):
                for mb in range(MB):
                    xmb = xmbp.tile([128, KT, 128], F32R, tag="xmb",
                                    name=f"xmb2_{mb}")
                    nc.gpsimd.dma_start(out=xmb[:],
                                        in_=xt_r[:, :, mb * 128:(mb + 1) * 128])
                    stgl = [stglp.tile([128, 1024], F32, tag="stgl",
                                       name=f"stgl_{mb}_{h}") for h in range(2)]
                    for h in range(2):
                        nc.scalar.dma_start(
                            out=stgl[h][:],
                            in_=stage_d[mb * 128:(mb + 1) * 128,
                                        h * 1024:(h + 1) * 1024])
                    ps_xs = pssp.tile([128, 2], F32, tag="pss",
                                      name=f"psxs{mb}")
                    pcs = [pcp.tile([128, 512], F32, tag="pc",
                                    name=f"p2_{mb}_{nb}") for nb in range(NB)]
                    for kt in range(KT):
                        nc.tensor.matmul(ps_xs[:], xmb[:, kt, :], ones_col[:],
                                         start=(kt == 0), stop=(kt == KT - 1))
                        if kt >= KH:
                            for nb in range(NB):
                                nc.tensor.matmul(
                                    pcs[nb][:], xmb[:, kt, :],
                                    geff[kt][:, nb * 512:(nb + 1) * 512],
                                    start=(kt == KH), stop=False)
                    beta = sp.tile([128, 1], F32, tag=f"beta{mb}",
                                   name=f"beta{mb}")
                    nc.vector.tensor_scalar(beta[:], ps_xs[:, 0:1], kap_b, None,
                                            ALU.mult)
                    for nb in range(NB):
                        nc.tensor.matmul(pcs[nb][:], ones_row[:],
                                         bias_row[:, nb * 512:(nb + 1) * 512],
                                         start=False, stop=True)
                        osb = osbp.tile([128, 512], F32, tag="osb",
                                        name=f"ep_{mb}_{nb}")
                        nc.vector.scalar_tensor_tensor(
                            osb[:], pcs[nb][:], beta[:],
                            stgl[nb // 2][:, (nb % 2) * 512:(nb % 2 + 1) * 512],
                            ALU.add, ALU.add)
                        nc.sync.dma_start(
                            out=out_d[mb * 128:(mb + 1) * 128,
                                      nb * 512:(nb + 1) * 512],
                            in_=osb[:])
    nc.finalize()
    return nc


def _prep_inputs(x, weight, bias):
    wtT = np.ascontiguousarray(weight.T)          # [K, N]
    wmin = float(wtT.min())
    wmax = float(wtT.max())
    s = (G_MAX - G_MIN) / (wmax - wmin)
    a = G_MIN / s - wmin
    kappa = wmin - G_MIN / s
    mmx = np.zeros((1, 20), dtype=np.float32)
    mmx[0, 0] = s
    mmx[0, 1] = a
    mmx[0, 2] = kappa
    mmx[0, 4:20] = [-256.0 * kt * s for kt in range(KT)]

    bias2 = np.ascontiguousarray(bias.reshape(1, N)).astype(np.float32)
    in_maps = []
    for c in range(N_CORES):
        x_c = x[c * BC:(c + 1) * BC, :]           # [BC, K]
        xt_c = np.ascontiguousarray(x_c.T)
        in_maps.append({"wt": wtT, "xt": xt_c, "bias": bias2, "mmx": mmx})
    return in_maps


def _run(x, weight, bias, trace=False, trace_kwargs=None):
    if "nc" not in _CACHE:
        _CACHE["nc"] = _build_nc()
    nc = _CACHE["nc"]
    in_maps = _prep_inputs(x, weight, bias)
    res = run_bass_kernel_spmd(nc, in_maps, list(range(N_CORES)), trace=trace,
                               **(trace_kwargs or {}))
    out = np.concatenate([res.results[c]["out"] for c in range(N_CORES)], axis=0)
    return out, res


def kernel(x, weight, bias):
    x = np.asarray(x, dtype=np.float32)
    weight = np.asarray(weight, dtype=np.float32)
    bias = np.asarray(bias, dtype=np.float32)
    out, _ = _run(x, weight, bias, trace=False)
    return out.astype(np.float32)


# revision 9
# speedup vs baseline: 1.3765x; 1.1270x over previous
"""Trainium2 Bass kernel for the memristor-crossbar layer (nn_CustomLayer_30588757082254).

out = unmap(x @ G_eff) + bias, where G_eff = 1/(1/G + R_par) is an elementwise
transform of weight.T with globally min/max-normalized conductances.

Strategy: data-parallel over batch (8 cores x 1024 rows). Each core receives the
full weight.T, its x-slice pre-transposed to [K, 1024] (layout-only host prep,
fed as fp32r bits), and bias. The conductance transform, the 1024x2048x2048
fp32r matmul, the row-sum correction and the bias add all run on device. Host
prep computes only data layout plus the two scalar weight statistics
(wmin/wmax -> s, a, kappa; ~0.01% of the FLOPs) that every transform op
depends on.

Math (S = 1/s folds the output unmapping scale into the transform for free):
  s = (g_max-g_min)/(wmax-wmin);  a = g_min/s - wmin
  S*G = WT + a
  w := u*s = recip(WT+a) + s*R          (R = colvec2 - 256*kt;  colvec2 = 4098+2n-2p)
  geff' := recip(w) = S*G_eff
  out = x@geff' + bias + xs*kappa       (kappa = wmin - g_min*S)

The K accumulation is split in two phases of 8 k-tiles so PSUM groups close as
soon as the first half of the transform is done: phase-1 partials are flushed
PSUM -> SBUF (ACT copy) -> DRAM stage, overlapping phase-2's transform; phase-2
re-loads the stage and the epilogue STT combines psum + beta + stage on DVE.
Row-sums (xs) run on the PE during the phase-1 transform window.
"""
import numpy as np

import concourse.bass as bass
import concourse.mybir as mybir
import concourse.tile as tile
from concourse import bacc
from concourse.bass_utils import run_bass_kernel_spmd
from concourse.dve_ops import RECIP_APPROX_FAST_CONSTS, RECIPROCAL_APPROX_FAST

F32 = mybir.dt.float32
F32R = mybir.dt.float32r
I32 = mybir.dt.int32
AF = mybir.ActivationFunctionType
ALU = mybir.AluOpType
AX = mybir.AxisListType
CRC = RECIP_APPROX_FAST_CONSTS

N_CORES = 8
B, K, N = 8192, 2048, 2048
BC = B // N_CORES            # 1024 batch rows per core
KT = K // 128                # 16 k-tiles
KH = KT // 2                 # k-tiles per phase
MB = BC // 128               # 8 m-blocks per core
NB = N // 512                # 4 psum column chunks
Q = 4                        # transform processed in column quarters of 512

PARASITIC_R = 2.0
G_MIN, G_MAX = 1.0 / 100000.0, 1.0 / 1000.0

_CACHE = {}


def _build_nc():
    nc = bacc.Bacc("TRN2", target_bir_lowering=False, debug=False,
                   num_devices=N_CORES)
    wt_in = nc.dram_tensor("wt", [K, N], F32, kind="ExternalInput")
    xt_in = nc.dram_tensor("xt", [K, BC], F32R, kind="ExternalInput")
    bias_in = nc.dram_tensor("bias", [1, N], F32R, kind="ExternalInput")
    mmx_in = nc.dram_tensor("mmx", [1, 20], F32, kind="ExternalInput")
    out_d = nc.dram_tensor("out", [BC, N], F32, kind="ExternalOutput")
    stage_d = nc.dram_tensor("stage", [BC, N], F32)

    xt_r = xt_in.rearrange("(kt p) m -> p kt m", p=128)

    with tile.TileContext(nc) as tc:
        with (
            tc.tile_pool(name="geffp", bufs=1) as geffp,
            tc.tile_pool(name="wtp", bufs=3) as wtp,
            tc.tile_pool(name="t1p", bufs=2) as t1p,
            tc.tile_pool(name="xmbp", bufs=3) as xmbp,
            tc.tile_pool(name="osbp", bufs=4) as osbp,
            tc.tile_pool(name="stglp", bufs=2) as stglp,
            tc.tile_pool(name="cvp", bufs=1) as cvp,
            tc.tile_pool(name="smallp", bufs=1) as sp,
            tc.tile_pool(name="pcp", bufs=6, space="PSUM") as pcp,
            tc.tile_pool(name="pssp", bufs=2, space="PSUM") as pssp,
        ):
            # -------- colvec2 integer part first (no runtime deps) --------
            cv2s = cvp.tile([128, N], F32, tag="cv2s")
            for q in range(Q):
                cvi = osbp.tile([128, 512], I32, tag="osb", name=f"cvi{q}")
                nc.gpsimd.iota(cvi[:], pattern=[[2, 512]],
                               base=4098 + 1024 * q, channel_multiplier=-2)
                nc.vector.tensor_copy(cv2s[:, q * 512:(q + 1) * 512], cvi[:])

            # ---------------- tiny inputs + broadcast ----------------
            with nc.named_scope("setup"):
                mmx = sp.tile([1, 20], F32, tag="mmx")
                nc.sync.dma_start(out=mmx[:], in_=mmx_in[:])
                bias_row = sp.tile([1, N], F32R, tag="bias_row")
                nc.sync.dma_start(out=bias_row[:], in_=bias_in[:])
                ones_f = sp.tile([1, 128], F32, tag="ones_f")
                nc.vector.memset(ones_f[:], 1.0)
                ps_bc = pssp.tile([128, 20], F32, tag="pss")
                nc.tensor.matmul(ps_bc[:], ones_f[:], mmx[:], start=True,
                                 stop=True)
                bcv = sp.tile([128, 20], F32, tag="bcv")
                nc.vector.tensor_copy(bcv[:], ps_bc[:])
                ones_col_f = sp.tile([128, 2], F32, tag="ones_col_f")
                nc.vector.memset(ones_col_f[:], 1.0)
                ones_col = sp.tile([128, 2], F32R, tag="ones_col")
                nc.vector.tensor_copy(ones_col[:], ones_col_f[:])
                ones_row_f = sp.tile([1, 128], F32, tag="ones_row_f")
                nc.vector.memset(ones_row_f[:], 1.0)
                ones_row = sp.tile([1, 128], F32R, tag="ones_row")
                nc.vector.tensor_copy(ones_row[:], ones_row_f[:])
            s_b = bcv[:, 0:1]
            a_b = bcv[:, 1:2]
            kap_b = bcv[:, 2:3]
            rk_s = bcv[:, 4:20]
            # scale colvec by s
            nc.vector.tensor_scalar(cv2s[:], cv2s[:], s_b, None, ALU.mult)

            # ---------------- W DMA stream (halves, shared slots) ------------
            wt_t = {}
            for kt in range(KT):
                for h in range(2):
                    w_t = wtp.tile([128, 1024], F32, tag="wt", name=f"wt{kt}_{h}")
                    nc.sync.dma_start(
                        out=w_t[:],
                        in_=wt_in[kt * 128:(kt + 1) * 128,
                                  h * 1024:(h + 1) * 1024])
                    wt_t[kt, h] = w_t

            xmbs = {}
            def transform_tile(kt):
                """geff'[kt] = recip(recip(WT+a) + s*R), in 512-col quarters."""
                ge = geffp.tile([128, N], F32R, tag=f"ge{kt}", name=f"ge{kt}")
                for h in range(2):
                    hs = h * 1024
                    t1 = t1p.tile([128, 1024], F32, tag="t1", name=f"t1_{kt}_{h}")
                    nc.scalar.activation(t1[:], wt_t[kt, h][:],
                                         AF.Identity, bias=a_b, scale=1.0)
                    nc.vector._custom_dve(RECIPROCAL_APPROX_FAST, out=t1[:],
                                          in0=t1[:], s0=CRC["s0"],
                                          s1=CRC["s1"], imm2=CRC["imm2"])
                    nc.vector.scalar_tensor_tensor(
                        t1[:], t1[:], rk_s[:, kt:kt + 1],
                        cv2s[:, hs:hs + 1024], ALU.add, ALU.add)
                    nc.vector._custom_dve(RECIPROCAL_APPROX_FAST,
                                          out=ge[:, hs:hs + 1024], in0=t1[:],
                                          s0=CRC["s0"], s1=CRC["s1"],
                                          imm2=CRC["imm2"])
                return ge

            # ---------------- phase 1 transform ----------------
            geff = {}
            with nc.named_scope("transform1"):
                for kt in range(KH):
                    geff[kt] = transform_tile(kt)

            # ------- phase 2 transform interleaved with phase-1 matmuls ------
            with nc.named_scope("p2t_mm1"):
                for j in range(MB):
                    geff[KH + j] = transform_tile(KH + j)
                    mb = j
                    xmb1 = xmbp.tile([128, KH, 128], F32R, tag="xmb",
                                     name=f"xmb1_{mb}")
                    nc.gpsimd.dma_start(
                        out=xmb1[:],
                        in_=xt_r[:, 0:KH, mb * 128:(mb + 1) * 128])
                    xmbs[1, mb] = xmb1
                    for nb in range(NB):
                        pc = pcp.tile([128, 512], F32, tag="pc",
                                      name=f"p1_{mb}_{nb}")
                        for kt in range(KH):
                            nc.tensor.matmul(
                                pc[:], xmbs[1, mb][:, kt, :],
                                geff[kt][:, nb * 512:(nb + 1) * 512],
                                start=(kt == 0), stop=(kt == KH - 1))
                        fl = osbp.tile([128, 512], F32, tag="osb",
                                       name=f"fl_{mb}_{nb}")
                        nc.scalar.copy(fl[:], pc[:])
                        nc.scalar.dma_start(
                            out=stage_d[mb * 128:(mb + 1) * 128,
                                        nb * 512:(nb + 1) * 512],
                            in_=fl[:])

            # ---------------- phase 2 matmuls + epilogue ----------------
            with nc.named_scope("mm2"):
                for mb in range(MB):
                    xmb = xmbp.tile([128, KT, 128], F32R, tag="xmb",
                                    name=f"xmb2_{mb}")
                    nc.gpsimd.dma_start(out=xmb[:],
                                        in_=xt_r[:, :, mb * 128:(mb + 1) * 128])
                    stgl = [stglp.tile([128, 1024], F32, tag="stgl",
                                       name=f"stgl_{mb}_{h}") for h in range(2)]
                    for h in range(2):
                        nc.scalar.dma_start(
                            out=stgl[h][:],
                            in_=stage_d[mb * 128:(mb + 1) * 128,
                                        h * 1024:(h + 1) * 1024])
                    ps_xs = pssp.tile([128, 2], F32, tag="pss",
                                      name=f"psxs{mb}")
                    pcs = [pcp.tile([128, 512], F32, tag="pc",
                                    name=f"p2_{mb}_{nb}") for nb in range(NB)]
                    for kt in range(KT):
                        nc.tensor.matmul(ps_xs[:], xmb[:, kt, :], ones_col[:],
                                         start=(kt == 0), stop=(kt == KT - 1))
                        if kt >= KH:
                            for nb in range(NB):
                                nc.tensor.matmul(
                                    pcs[nb][:], xmb[:, kt, :],
                                    geff[kt][:, nb * 512:(nb + 1) * 512],
                                    start=(kt == KH), stop=False)
                    beta = sp.tile([128, 1], F32, tag=f"beta{mb}",
                                   name=f"beta{mb}")
                    nc.vector.tensor_scalar(beta[:], ps_xs[:, 0:1], kap_b, None,
                                            ALU.mult)
                    for nb in range(NB):
                        nc.tensor.matmul(pcs[nb][:], ones_row[:],
                                         bias_row[:, nb * 512:(nb + 1) * 512],
                                         start=False, stop=True)
                        osb = osbp.tile([128, 512], F32, tag="osb",
                                        name=f"ep_{mb}_{nb}")
                        nc.vector.scalar_tensor_tensor(
                            osb[:], pcs[nb][:], beta[:],
                            stgl[nb // 2][:, (nb % 2) * 512:(nb % 2 + 1) * 512],
                            ALU.add, ALU.add)
                        nc.sync.dma_start(
                            out=out_d[mb * 128:(mb + 1) * 128,
                                      nb * 512:(nb + 1) * 512],
                            in_=osb[:])
    nc.finalize()
    return nc


def _prep_inputs(x, weight, bias):
    wtT = np.ascontiguousarray(weight.T)          # [K, N]
    wmin = float(wtT.min())
    wmax = float(wtT.max())
    s = (G_MAX - G_MIN) / (wmax - wmin)
    a = G_MIN / s - wmin
    kappa = wmin - G_MIN / s
    mmx = np.zeros((1, 20), dtype=np.float32)
    mmx[0, 0] = s
    mmx[0, 1] = a
    mmx[0, 2] = kappa
    mmx[0, 4:20] = [-256.0 * kt * s for kt in range(KT)]

    bias2 = np.ascontiguousarray(bias.reshape(1, N)).astype(np.float32)
    in_maps = []
    for c in range(N_CORES):
        x_c = x[c * BC:(c + 1) * BC, :]           # [BC, K]
        xt_c = np.ascontiguousarray(x_c.T)
        in_maps.append({"wt": wtT, "xt": xt_c, "bias": bias2, "mmx": mmx})
    return in_maps


def _run(x, weight, bias, trace=False, trace_kwargs=None):
    if "nc" not in _CACHE:
        _CACHE["nc"] = _build_nc()
    nc = _CACHE["nc"]
    in_maps = _prep_inputs(x, weight, bias)
    res = run_bass_kernel_spmd(nc, in_maps, list(range(N_CORES)), trace=trace,
                               **(trace_kwargs or {}))
    out = np.concatenate([res.results[c]["out"] for c in range(N_CORES)], axis=0)
    return out, res


def kernel(x, weight, bias):
    x = np.asarray(x, dtype=np.float32)
    weight = np.asarray(weight, dtype=np.float32)
    bias = np.asarray(bias, dtype=np.float32)
    out, _ = _run(x, weight, bias, trace=False)
    return out.astype(np.float32)
